# revision 26
# baseline (speedup 1.0000x reference)
"""RWKV WKV attention kernel for TRN2 (Bass/Tile), batch-parallel over 8 cores.

v3: all-bf16 matmul path, single-pass (no DRAM scratch), DMA-transposed x
loads (no PE transposes), exp-rescaled recurrence (one ACT exp per step).

Per core (one batch element), chunked over TC=1024, e-groups of 4:
  mixes:  z = x_t - x_{t-1}; xm* = z*m_* + x_{t-1}        (DVE, bf16)
  GEMMs:  k/v/r = W_* @ xm*  (PE, bf16, [d,t] layout)
  wkv:    ek = exp(k) (ACT); a = ek*v (DVE); sa/sb = decaying scans over
          shifted a/ek (DVE + GpSimd); num = c*sa + a, den = c*sb + ek with
          c = exp(-u) (in-place); wkv = num/den; rw = (tanh(r/2)+1)*wkv
  out:    o = rw^T @ (0.5*Wo)^T  (PE), fp32 DMA straight from PSUM.

Host-packed weights [128, 8*1024] bf16: arr[p, j*1024+e] = W[e, j*128+p].
cv fp32 [128, 40] (col j of each group = channels j*128..j*128+127):
  0-7 mk, 8-15 mv, 16-23 mr, 24-31 ew=exp(-exp(time_decay)), 32-39 c=e^-u.
xp bf16 [16+T, D]: 16 zero rows then x (halo for the shifted time mix).
"""
import sys
for p in ("/opt/trn_rl_repo",):
    if p not in sys.path:
        sys.path.insert(0, p)

import numpy as np
from contextlib import ExitStack

import concourse.bass as bass
import concourse.tile as tile
from concourse import bacc, mybir

dt = mybir.dt
AF = mybir.ActivationFunctionType
OP = mybir.AluOpType

D = 1024
NJ = D // 128  # 8 channel chunks
GPSIMD_SCAN = False
STUB_EW = 0  # 0=full, 1=skip wkv/rw, 2=skip num..rw, 3=skip all
XPOSE_ENG = "sync"  # which HWDGE engine issues dma_start_transpose


def build(nc, T=4096, TC=1024):
    nch = T // TC
    H = 512           # PSUM-granularity half-tiles
    NH = TC // H
    EG = 4            # e-group size

    XP = nc.dram_tensor("xp", [16 + T, D], dt.bfloat16, kind="ExternalInput").ap()
    WK = nc.dram_tensor("wk", [128, NJ * D], dt.bfloat16, kind="ExternalInput").ap()
    WV = nc.dram_tensor("wv", [128, NJ * D], dt.bfloat16, kind="ExternalInput").ap()
    WR = nc.dram_tensor("wr", [128, NJ * D], dt.bfloat16, kind="ExternalInput").ap()
    WO = nc.dram_tensor("wo", [128, NJ * D], dt.bfloat16, kind="ExternalInput").ap()
    CV = nc.dram_tensor("cv", [128, 40], dt.float32, kind="ExternalInput").ap()
    CVH = nc.dram_tensor("cvh", [128, 40], dt.bfloat16, kind="ExternalInput").ap()
    O = nc.dram_tensor("o", [T, D], dt.bfloat16, kind="ExternalOutput").ap()

    with tile.TileContext(nc) as tc, ExitStack() as ctx:
        wpool = ctx.enter_context(tc.tile_pool(name="wpool", bufs=1))
        xtp = ctx.enter_context(tc.tile_pool(name="xtp", bufs=7))
        zp = ctx.enter_context(tc.tile_pool(name="zp", bufs=8))
        kxp = ctx.enter_context(tc.tile_pool(name="kxp", bufs=8))
        vxp = ctx.enter_context(tc.tile_pool(name="vxp", bufs=8))
        rxp = ctx.enter_context(tc.tile_pool(name="rxp", bufs=8))
        kvps = ctx.enter_context(tc.tile_pool(name="kvps", bufs=4, space="PSUM"))
        rps = ctx.enter_context(tc.tile_pool(name="rps", bufs=2, space="PSUM"))
        ops_ = ctx.enter_context(tc.tile_pool(name="ops", bufs=2, space="PSUM"))
        ekp = ctx.enter_context(tc.tile_pool(name="ekp", bufs=4))
        vcp = ctx.enter_context(tc.tile_pool(name="vcp", bufs=2))
        ap_ = ctx.enter_context(tc.tile_pool(name="ap", bufs=4))
        srp = ctx.enter_context(tc.tile_pool(name="srp", bufs=4))
        sap = ctx.enter_context(tc.tile_pool(name="sap", bufs=2))
        sbp = ctx.enter_context(tc.tile_pool(name="sbp", bufs=2))
        dnp = ctx.enter_context(tc.tile_pool(name="dnp", bufs=2))
        rwp = ctx.enter_context(tc.tile_pool(name="rwp", bufs=9))
        ocp = ctx.enter_context(tc.tile_pool(name="ocp", bufs=2))
        stp = ctx.enter_context(tc.tile_pool(name="stp", bufs=1))

        wk_t = wpool.tile([128, NJ * D], dt.bfloat16, tag="wk")
        nc.sync.dma_start(wk_t[:], WK)
        wv_t = wpool.tile([128, NJ * D], dt.bfloat16, tag="wv")
        nc.sync.dma_start(wv_t[:], WV)
        wr_t = wpool.tile([128, NJ * D], dt.bfloat16, tag="wr")
        nc.sync.dma_start(wr_t[:], WR)
        wo_t = wpool.tile([128, NJ * D], dt.bfloat16, tag="wo")
        nc.sync.dma_start(wo_t[:], WO)
        cv = wpool.tile([128, 40], dt.float32, tag="cv")
        nc.sync.dma_start(cv[:], CV)
        cvh = wpool.tile([128, 40], dt.bfloat16, tag="cvh")
        nc.sync.dma_start(cvh[:], CVH)

        def states(prefix, dtype):
            ts_ = []
            for e in range(NJ):
                t = stp.tile([128, 1], dtype, tag=f"{prefix}{e}")
                nc.vector.memset(t[:], 0.0)
                ts_.append(t)
            return ts_

        ekst = states("ekst", dt.bfloat16)
        ast = states("ast", dt.bfloat16)
        alst = states("alst", dt.float32)
        best = states("best", dt.float32)

        def load_x(c):
            t0 = c * TC
            xts = []
            for j in range(NJ):
                xt = xtp.tile([128, TC + 16], dt.bfloat16, tag="xt")
                xeng = nc.sync if XPOSE_ENG == "sync" else nc.scalar
                xeng.dma_start_transpose(
                    xt[:], XP[t0: t0 + TC + 16, j * 128:(j + 1) * 128])
                xts.append(xt)
            return xts

        def mix_one(xts, pool, col0, zs):
            """One projection's time-mix for all j: z*m + x_prev, with
            z = x_t - x_prev computed once (GpSimd) and shared via zs."""
            ms = []
            for j in range(NJ):
                xt = xts[j]
                if j not in zs:
                    z = zp.tile([128, TC], dt.bfloat16, tag="z")
                    nc.vector.tensor_tensor(
                        z[:], xt[:, 16:TC + 16], xt[:, 15:TC + 15],
                        OP.subtract)
                    zs[j] = z
                m = pool.tile([128, TC], dt.bfloat16, tag="m")
                nc.vector.scalar_tensor_tensor(
                    m[:], zs[j][:], cvh[:, col0 + j: col0 + j + 1],
                    xt[:, 15:TC + 15], OP.mult, OP.add)
                ms.append(m)
            return ms

        def gemm(w_t, xm, e, h, pool):
            acc = pool.tile([128, H], dt.float32, tag="acc")
            for j in range(NJ):
                nc.tensor.matmul(
                    acc[:], w_t[:, j * D + e * 128: j * D + (e + 1) * 128],
                    xm[j][:, h * H:(h + 1) * H],
                    start=(j == 0), stop=(j == NJ - 1))
            return acc

        def out_gemm(prev):
            rws, c = prev
            t0 = c * TC
            for ts in range(TC // 128):
                for eh in range(D // H):
                    pso = ops_.tile([128, H], dt.float32, tag="pso")
                    for j in range(NJ):
                        nc.tensor.matmul(
                            pso[:], rws[j][:, ts * 128:(ts + 1) * 128],
                            wo_t[:, j * D + eh * H: j * D + (eh + 1) * H],
                            start=(j == 0), stop=(j == NJ - 1))
                    oc = ocp.tile([128, H], dt.bfloat16, tag="oc")
                    nc.scalar.copy(oc[:], pso[:])
                    nc.sync.dma_start(
                        O[t0 + ts * 128: t0 + (ts + 1) * 128,
                          eh * H:(eh + 1) * H], oc[:])

        # prologue: x + mixes for chunk 0
        xts = load_x(0)
        zs = {}
        xmk = mix_one(xts, kxp, 0, zs)
        xmv = mix_one(xts, vxp, 8, zs)
        xmr = mix_one(xts, rxp, 16, zs)
        prev_o = None  # (rws, c) pending output GEMM

        for c in range(nch):
            last = c + 1 >= nch
            if not last:
                xts_n = load_x(c + 1)
            rws_c = []
            for g in range(NJ // EG):
                es = range(g * EG, (g + 1) * EG)
                eks, as_, srs = {}, {}, {}
                # ---- k phase ----
                for e in es:
                    ek = ekp.tile([128, TC + 1], dt.bfloat16, tag="ek")
                    nc.scalar.copy(ek[:, 0:1], ekst[e][:])
                    for h in range(NH):
                        acc = gemm(wk_t, xmk, e, h, kvps)
                        nc.scalar.activation(
                            ek[:, 1 + h * H: 1 + (h + 1) * H], acc[:], AF.Exp)
                    nc.scalar.copy(ekst[e][:], ek[:, TC:TC + 1])
                    eks[e] = ek
                if g == 1 and not last:
                    zs_n = {}
                    xmk_n = mix_one(xts_n, kxp, 0, zs_n)
                # ---- v phase (+ a = ek*v) ----
                for e in es:
                    vc = vcp.tile([128, TC], dt.bfloat16, tag="vc")
                    for h in range(NH):
                        acc = gemm(wv_t, xmv, e, h, kvps)
                        nc.scalar.copy(vc[:, h * H:(h + 1) * H], acc[:])
                    a = ap_.tile([128, TC + 1], dt.bfloat16, tag="a")
                    nc.vector.tensor_copy(a[:, 0:1], ast[e][:])
                    nc.gpsimd.tensor_tensor(
                        a[:, 1:TC + 1], eks[e][:, 1:TC + 1], vc[:], OP.mult)
                    nc.vector.tensor_copy(ast[e][:], a[:, TC:TC + 1])
                    as_[e] = a
                if g == 1 and not last:
                    xmv_n = mix_one(xts_n, vxp, 8, zs_n)
                # ---- r phase ----
                for e in es:
                    sr = srp.tile([128, TC], dt.bfloat16, tag="sr")
                    for h in range(NH):
                        acc = gemm(wr_t, xmr, e, h, rps)
                        nc.scalar.activation(
                            sr[:, h * H:(h + 1) * H], acc[:], AF.Sigmoid)
                    srs[e] = sr
                if g == 1 and not last:
                    xmr_n = mix_one(xts_n, rxp, 16, zs_n)
                # ---- wkv elementwise chain ----
                for e in es:
                    ek, a = eks[e], as_[e]
                    ewb = cvh[:, 24 + e: 25 + e].broadcast_to([128, TC])
                    ce = cv[:, 32 + e: 33 + e]
                    sa = sap.tile([128, TC], dt.bfloat16, tag="sa")
                    nc.vector.tensor_tensor_scan(
                        sa[:], ewb, a[:, 0:TC], alst[e][:], OP.mult, OP.add)
                    nc.vector.tensor_copy(alst[e][:], sa[:, TC - 1:TC])
                    sb = sbp.tile([128, TC], dt.bfloat16, tag="sb")
                    nc.vector.tensor_tensor_scan(
                        sb[:], ewb, ek[:, 0:TC], best[e][:], OP.mult, OP.add)
                    nc.vector.tensor_copy(best[e][:], sb[:, TC - 1:TC])
                    # num = c*sa + a (in-place), den = c*sb + ek; AP
                    # scalars are DVE-only (Pool rejects scalar ptrs).
                    nc.vector.scalar_tensor_tensor(
                        sa[:], sa[:], ce, a[:, 1:TC + 1], OP.mult, OP.add)
                    den = dnp.tile([128, TC], dt.float32, tag="den")
                    cb = cvh[:, 32 + e: 33 + e].broadcast_to([128, TC])
                    nc.gpsimd.tensor_tensor(den[:], sb[:], cb, OP.mult)
                    nc.gpsimd.tensor_tensor(den[:], den[:], ek[:, 1:TC + 1],
                                            OP.add)
                    nc.vector.reciprocal_approx_fast(den[:], den[:])
                    nc.gpsimd.tensor_tensor(sa[:], sa[:], den[:], OP.mult)
                    rw = rwp.tile([128, TC], dt.bfloat16, tag="rw")
                    nc.gpsimd.tensor_tensor(rw[:], srs[e][:], sa[:], OP.mult)
                    rws_c.append(rw)
                if g == 0 and prev_o is not None:
                    out_gemm(prev_o)
            prev_o = (rws_c, c)
            if not last:
                xts, xmk, xmv, xmr = xts_n, xmk_n, xmv_n, xmr_n

        out_gemm(prev_o)


def pack_inputs(x_slice, time_decay, time_first, time_mix_k, time_mix_v,
                time_mix_r, Wk, Wv, Wr, Wo):
    """Host-side packing for one core. x_slice: [T, D] fp32."""
    import ml_dtypes
    bf16 = ml_dtypes.bfloat16

    def packw(W):
        return np.ascontiguousarray(
            W.T.reshape(NJ, 128, D).transpose(1, 0, 2).reshape(128, NJ * D)
        ).astype(bf16)

    def packv(v):
        return np.ascontiguousarray(v.reshape(NJ, 128).T).astype(np.float32)

    T = x_slice.shape[0]
    xp = np.zeros((16 + T, D), dtype=bf16)
    xp[16:] = x_slice.astype(bf16)

    mk = time_mix_k.reshape(D).astype(np.float32)
    mv = time_mix_v.reshape(D).astype(np.float32)
    mr = time_mix_r.reshape(D).astype(np.float32)
    ew = np.exp(-np.exp(time_decay.astype(np.float32))).astype(np.float32)
    u = time_first.astype(np.float32).reshape(D)
    cvals = np.concatenate([
        packv(mk), packv(mv), packv(mr),
        packv(ew), packv(np.exp(-u))], axis=1).astype(np.float32)
    return {
        "xp": xp,
        "wk": packw(Wk), "wv": packw(Wv), "wr": packw(Wr),
        "wo": packw(Wo),
        "cv": cvals, "cvh": cvals.astype(bf16),
    }


# ---------------------------------------------------------------------------
# Harness entry point: full inputs in, full output out, 8-way batch-parallel.
# ---------------------------------------------------------------------------
_CACHE = {}
_last_exec_time_ns = None


def _get_program(n_cores):
    key = ("prog", n_cores)
    if key not in _CACHE:
        nc = bacc.Bacc("TRN2", target_bir_lowering=False, debug=False,
                       num_devices=n_cores)
        build(nc, T=4096)
        nc.compile()
        _CACHE[key] = nc
    return _CACHE[key]


def kernel(x, time_decay, time_first, time_mix_k, time_mix_v, time_mix_r,
           Wk, Wv, Wr, Wo):
    """WKV attention: x [8, 4096, 1024] fp32 -> out [8, 4096, 1024] fp32.

    Shards batch across the 8 NeuronCores (one batch element per core).
    """
    global _last_exec_time_ns
    import os
    import ml_dtypes
    from concourse import bass_utils

    x = np.asarray(x, dtype=np.float32)
    B = x.shape[0]
    base = pack_inputs(x[0], np.asarray(time_decay), np.asarray(time_first),
                       np.asarray(time_mix_k), np.asarray(time_mix_v),
                       np.asarray(time_mix_r), np.asarray(Wk), np.asarray(Wv),
                       np.asarray(Wr), np.asarray(Wo))
    in_maps = []
    for b in range(B):
        m = dict(base)
        if b > 0:
            xp = np.zeros_like(base["xp"])
            xp[16:] = x[b].astype(ml_dtypes.bfloat16)
            m["xp"] = xp
        in_maps.append(m)

    nc = _get_program(B)
    trace = os.environ.get("WKV_TRACE", "0") == "1"
    r = bass_utils.run_bass_kernel_spmd(nc, in_maps, core_ids=list(range(B)),
                                        trace=trace)
    _last_exec_time_ns = r.exec_time_ns
    return np.stack([np.asarray(r.results[b]["o"]).astype(np.float32)
                     for b in range(B)])


# revision 27
# speedup vs baseline: 1.1054x; 1.1054x over previous
"""RWKV WKV attention kernel for TRN2 (Bass/Tile), batch-parallel over 8 cores.

v3: all-bf16 matmul path, single-pass (no DRAM scratch), DMA-transposed x
loads (no PE transposes), exp-rescaled recurrence (one ACT exp per step).

Per core (one batch element), chunked over TC=1024, e-groups of 4:
  mixes:  z = x_t - x_{t-1}; xm* = z*m_* + x_{t-1}        (DVE, bf16)
  GEMMs:  k/v/r = W_* @ xm*  (PE, bf16, [d,t] layout)
  wkv:    ek = exp(k) (ACT); a = ek*v (DVE); sa/sb = decaying scans over
          shifted a/ek (DVE + GpSimd); num = c*sa + a, den = c*sb + ek with
          c = exp(-u) (in-place); wkv = num/den; rw = (tanh(r/2)+1)*wkv
  out:    o = rw^T @ (0.5*Wo)^T  (PE), fp32 DMA straight from PSUM.

Host-packed weights [128, 8*1024] bf16: arr[p, j*1024+e] = W[e, j*128+p].
cv fp32 [128, 40] (col j of each group = channels j*128..j*128+127):
  0-7 mk, 8-15 mv, 16-23 mr, 24-31 ew=exp(-exp(time_decay)), 32-39 c=e^-u.
xp bf16 [16+T, D]: 16 zero rows then x (halo for the shifted time mix).
"""
import sys
for p in ("/opt/trn_rl_repo",):
    if p not in sys.path:
        sys.path.insert(0, p)

import numpy as np
from contextlib import ExitStack

import concourse.bass as bass
import concourse.tile as tile
from concourse import bacc, mybir

dt = mybir.dt
AF = mybir.ActivationFunctionType
OP = mybir.AluOpType

D = 1024
NJ = D // 128  # 8 channel chunks
GPSIMD_SCAN = False
STUB_EW = 0  # 0=full, 1=skip wkv/rw, 2=skip num..rw, 3=skip all
XPOSE_ENG = "sync"  # which HWDGE engine issues dma_start_transpose


def build(nc, T=4096, TC=1024):
    nch = T // TC
    H = 512           # PSUM-granularity half-tiles
    NH = TC // H
    EG = 4            # e-group size

    XP = nc.dram_tensor("xp", [16 + T, D], dt.bfloat16, kind="ExternalInput").ap()
    WK = nc.dram_tensor("wk", [128, NJ * D], dt.bfloat16, kind="ExternalInput").ap()
    WV = nc.dram_tensor("wv", [128, NJ * D], dt.bfloat16, kind="ExternalInput").ap()
    WR = nc.dram_tensor("wr", [128, NJ * D], dt.bfloat16, kind="ExternalInput").ap()
    WO = nc.dram_tensor("wo", [128, NJ * D], dt.bfloat16, kind="ExternalInput").ap()
    CV = nc.dram_tensor("cv", [128, 40], dt.float32, kind="ExternalInput").ap()
    CVH = nc.dram_tensor("cvh", [128, 40], dt.bfloat16, kind="ExternalInput").ap()
    O = nc.dram_tensor("o", [T, D], dt.bfloat16, kind="ExternalOutput").ap()

    with tile.TileContext(nc) as tc, ExitStack() as ctx:
        wpool = ctx.enter_context(tc.tile_pool(name="wpool", bufs=1))
        xtp = ctx.enter_context(tc.tile_pool(name="xtp", bufs=7))
        zp = ctx.enter_context(tc.tile_pool(name="zp", bufs=8))
        kxp = ctx.enter_context(tc.tile_pool(name="kxp", bufs=8))
        vxp = ctx.enter_context(tc.tile_pool(name="vxp", bufs=8))
        rxp = ctx.enter_context(tc.tile_pool(name="rxp", bufs=8))
        kvps = ctx.enter_context(tc.tile_pool(name="kvps", bufs=4, space="PSUM"))
        rps = ctx.enter_context(tc.tile_pool(name="rps", bufs=2, space="PSUM"))
        ops_ = ctx.enter_context(tc.tile_pool(name="ops", bufs=2, space="PSUM"))
        ekp = ctx.enter_context(tc.tile_pool(name="ekp", bufs=4))
        vcp = ctx.enter_context(tc.tile_pool(name="vcp", bufs=2))
        ap_ = ctx.enter_context(tc.tile_pool(name="ap", bufs=4))
        srp = ctx.enter_context(tc.tile_pool(name="srp", bufs=4))
        sap = ctx.enter_context(tc.tile_pool(name="sap", bufs=2))
        sbp = ctx.enter_context(tc.tile_pool(name="sbp", bufs=2))
        dnp = ctx.enter_context(tc.tile_pool(name="dnp", bufs=2))
        rwp = ctx.enter_context(tc.tile_pool(name="rwp", bufs=9))
        ocp = ctx.enter_context(tc.tile_pool(name="ocp", bufs=2))
        stp = ctx.enter_context(tc.tile_pool(name="stp", bufs=1))

        wk_t = wpool.tile([128, NJ * D], dt.bfloat16, tag="wk")
        nc.sync.dma_start(wk_t[:], WK)
        wv_t = wpool.tile([128, NJ * D], dt.bfloat16, tag="wv")
        nc.sync.dma_start(wv_t[:], WV)
        wr_t = wpool.tile([128, NJ * D], dt.bfloat16, tag="wr")
        nc.sync.dma_start(wr_t[:], WR)
        wo_t = wpool.tile([128, NJ * D], dt.bfloat16, tag="wo")
        nc.sync.dma_start(wo_t[:], WO)
        cv = wpool.tile([128, 40], dt.float32, tag="cv")
        nc.sync.dma_start(cv[:], CV)
        cvh = wpool.tile([128, 40], dt.bfloat16, tag="cvh")
        nc.sync.dma_start(cvh[:], CVH)

        def states(prefix, dtype):
            ts_ = []
            for e in range(NJ):
                t = stp.tile([128, 1], dtype, tag=f"{prefix}{e}")
                nc.vector.memset(t[:], 0.0)
                ts_.append(t)
            return ts_

        ekst = states("ekst", dt.bfloat16)
        ast = states("ast", dt.bfloat16)
        alst = states("alst", dt.float32)
        best = states("best", dt.float32)

        def load_x(c):
            t0 = c * TC
            xts = []
            for j in range(NJ):
                xt = xtp.tile([128, TC + 16], dt.bfloat16, tag="xt")
                xeng = nc.sync if XPOSE_ENG == "sync" else nc.scalar
                xeng.dma_start_transpose(
                    xt[:], XP[t0: t0 + TC + 16, j * 128:(j + 1) * 128])
                xts.append(xt)
            return xts

        def mix_one(xts, pool, col0, zs):
            """One projection's time-mix for all j: z*m + x_prev, with
            z = x_t - x_prev computed once (GpSimd) and shared via zs."""
            ms = []
            for j in range(NJ):
                xt = xts[j]
                if j not in zs:
                    z = zp.tile([128, TC], dt.bfloat16, tag="z")
                    nc.vector.tensor_tensor(
                        z[:], xt[:, 16:TC + 16], xt[:, 15:TC + 15],
                        OP.subtract)
                    zs[j] = z
                m = pool.tile([128, TC], dt.bfloat16, tag="m")
                nc.vector.scalar_tensor_tensor(
                    m[:], zs[j][:], cvh[:, col0 + j: col0 + j + 1],
                    xt[:, 15:TC + 15], OP.mult, OP.add)
                ms.append(m)
            return ms

        def gemm(w_t, xm, e, h, pool):
            acc = pool.tile([128, H], dt.float32, tag="acc")
            for j in range(NJ):
                nc.tensor.matmul(
                    acc[:], w_t[:, j * D + e * 128: j * D + (e + 1) * 128],
                    xm[j][:, h * H:(h + 1) * H],
                    start=(j == 0), stop=(j == NJ - 1))
            return acc

        def out_gemm(prev):
            rws, c = prev
            t0 = c * TC
            for ts in range(TC // 128):
                for eh in range(D // H):
                    pso = ops_.tile([128, H], dt.float32, tag="pso")
                    for j in range(NJ):
                        nc.tensor.matmul(
                            pso[:], rws[j][:, ts * 128:(ts + 1) * 128],
                            wo_t[:, j * D + eh * H: j * D + (eh + 1) * H],
                            start=(j == 0), stop=(j == NJ - 1))
                    oc = ocp.tile([128, H], dt.bfloat16, tag="oc")
                    nc.scalar.copy(oc[:], pso[:])
                    nc.sync.dma_start(
                        O[t0 + ts * 128: t0 + (ts + 1) * 128,
                          eh * H:(eh + 1) * H], oc[:])

        # prologue: x + mixes for chunk 0
        xts = load_x(0)
        zs = {}
        xmk = mix_one(xts, kxp, 0, zs)
        xmv = mix_one(xts, vxp, 8, zs)
        xmr = mix_one(xts, rxp, 16, zs)
        prev_o = None  # (rws, c) pending output GEMM

        for c in range(nch):
            last = c + 1 >= nch
            if not last:
                xts_n = load_x(c + 1)
            rws_c = []
            for g in range(NJ // EG):
                es = range(g * EG, (g + 1) * EG)
                eks, as_, srs = {}, {}, {}
                # ---- k phase ----
                for e in es:
                    ek = ekp.tile([128, TC + 1], dt.bfloat16, tag="ek")
                    nc.scalar.copy(ek[:, 0:1], ekst[e][:])
                    for h in range(NH):
                        acc = gemm(wk_t, xmk, e, h, kvps)
                        nc.scalar.activation(
                            ek[:, 1 + h * H: 1 + (h + 1) * H], acc[:], AF.Exp)
                    nc.scalar.copy(ekst[e][:], ek[:, TC:TC + 1])
                    eks[e] = ek
                if g == 1 and not last:
                    zs_n = {}
                    xmk_n = mix_one(xts_n, kxp, 0, zs_n)
                # ---- v phase (+ a = ek*v) ----
                for e in es:
                    vc = vcp.tile([128, TC], dt.bfloat16, tag="vc")
                    for h in range(NH):
                        acc = gemm(wv_t, xmv, e, h, kvps)
                        nc.scalar.copy(vc[:, h * H:(h + 1) * H], acc[:])
                    a = ap_.tile([128, TC + 1], dt.bfloat16, tag="a")
                    nc.vector.tensor_copy(a[:, 0:1], ast[e][:])
                    nc.gpsimd.tensor_tensor(
                        a[:, 1:TC + 1], eks[e][:, 1:TC + 1], vc[:], OP.mult)
                    nc.vector.tensor_copy(ast[e][:], a[:, TC:TC + 1])
                    as_[e] = a
                if g == 1 and not last:
                    xmv_n = mix_one(xts_n, vxp, 8, zs_n)
                # ---- r phase ----
                for e in es:
                    sr = srp.tile([128, TC], dt.bfloat16, tag="sr")
                    for h in range(NH):
                        acc = gemm(wr_t, xmr, e, h, rps)
                        nc.scalar.activation(
                            sr[:, h * H:(h + 1) * H], acc[:], AF.Sigmoid)
                    srs[e] = sr
                if g == 1 and not last:
                    xmr_n = mix_one(xts_n, rxp, 16, zs_n)
                # ---- wkv elementwise chain ----
                for e in es:
                    ek, a = eks[e], as_[e]
                    ewb = cvh[:, 24 + e: 25 + e].broadcast_to([128, TC])
                    ce = cv[:, 32 + e: 33 + e]
                    sa = sap.tile([128, TC], dt.bfloat16, tag="sa")
                    nc.vector.tensor_tensor_scan(
                        sa[:], ewb, a[:, 0:TC], alst[e][:], OP.mult, OP.add)
                    nc.vector.tensor_copy(alst[e][:], sa[:, TC - 1:TC])
                    sb = sbp.tile([128, TC], dt.bfloat16, tag="sb")
                    nc.vector.tensor_tensor_scan(
                        sb[:], ewb, ek[:, 0:TC], best[e][:], OP.mult, OP.add)
                    nc.vector.tensor_copy(best[e][:], sb[:, TC - 1:TC])
                    # num = c*sa + a (in-place), den = c*sb + ek; AP
                    # scalars are DVE-only (Pool rejects scalar ptrs).
                    nc.vector.scalar_tensor_tensor(
                        sa[:], sa[:], ce, a[:, 1:TC + 1], OP.mult, OP.add)
                    den = dnp.tile([128, TC], dt.float32, tag="den")
                    nc.vector.scalar_tensor_tensor(
                        den[:], sb[:], ce, ek[:, 1:TC + 1], OP.mult, OP.add)
                    nc.vector.reciprocal_approx_fast(den[:], den[:])
                    nc.gpsimd.tensor_tensor(sa[:], sa[:], den[:], OP.mult)
                    rw = rwp.tile([128, TC], dt.bfloat16, tag="rw")
                    nc.gpsimd.tensor_tensor(rw[:], srs[e][:], sa[:], OP.mult)
                    rws_c.append(rw)
                if g == 0 and prev_o is not None:
                    out_gemm(prev_o)
            prev_o = (rws_c, c)
            if not last:
                xts, xmk, xmv, xmr = xts_n, xmk_n, xmv_n, xmr_n

        out_gemm(prev_o)


def pack_inputs(x_slice, time_decay, time_first, time_mix_k, time_mix_v,
                time_mix_r, Wk, Wv, Wr, Wo):
    """Host-side packing for one core. x_slice: [T, D] fp32."""
    import ml_dtypes
    bf16 = ml_dtypes.bfloat16

    def packw(W):
        return np.ascontiguousarray(
            W.T.reshape(NJ, 128, D).transpose(1, 0, 2).reshape(128, NJ * D)
        ).astype(bf16)

    def packv(v):
        return np.ascontiguousarray(v.reshape(NJ, 128).T).astype(np.float32)

    T = x_slice.shape[0]
    xp = np.zeros((16 + T, D), dtype=bf16)
    xp[16:] = x_slice.astype(bf16)

    mk = time_mix_k.reshape(D).astype(np.float32)
    mv = time_mix_v.reshape(D).astype(np.float32)
    mr = time_mix_r.reshape(D).astype(np.float32)
    ew = np.exp(-np.exp(time_decay.astype(np.float32))).astype(np.float32)
    u = time_first.astype(np.float32).reshape(D)
    cvals = np.concatenate([
        packv(mk), packv(mv), packv(mr),
        packv(ew), packv(np.exp(-u))], axis=1).astype(np.float32)
    return {
        "xp": xp,
        "wk": packw(Wk), "wv": packw(Wv), "wr": packw(Wr),
        "wo": packw(Wo),
        "cv": cvals, "cvh": cvals.astype(bf16),
    }


# ---------------------------------------------------------------------------
# Harness entry point: full inputs in, full output out, 8-way batch-parallel.
# ---------------------------------------------------------------------------
_CACHE = {}
_last_exec_time_ns = None


def _get_program(n_cores):
    key = ("prog", n_cores)
    if key not in _CACHE:
        nc = bacc.Bacc("TRN2", target_bir_lowering=False, debug=False,
                       num_devices=n_cores)
        build(nc, T=4096)
        nc.compile()
        _CACHE[key] = nc
    return _CACHE[key]


def kernel(x, time_decay, time_first, time_mix_k, time_mix_v, time_mix_r,
           Wk, Wv, Wr, Wo):
    """WKV attention: x [8, 4096, 1024] fp32 -> out [8, 4096, 1024] fp32.

    Shards batch across the 8 NeuronCores (one batch element per core).
    """
    global _last_exec_time_ns
    import os
    import ml_dtypes
    from concourse import bass_utils

    x = np.asarray(x, dtype=np.float32)
    B = x.shape[0]
    base = pack_inputs(x[0], np.asarray(time_decay), np.asarray(time_first),
                       np.asarray(time_mix_k), np.asarray(time_mix_v),
                       np.asarray(time_mix_r), np.asarray(Wk), np.asarray(Wv),
                       np.asarray(Wr), np.asarray(Wo))
    in_maps = []
    for b in range(B):
        m = dict(base)
        if b > 0:
            xp = np.zeros_like(base["xp"])
            xp[16:] = x[b].astype(ml_dtypes.bfloat16)
            m["xp"] = xp
        in_maps.append(m)

    nc = _get_program(B)
    trace = os.environ.get("WKV_TRACE", "0") == "1"
    r = bass_utils.run_bass_kernel_spmd(nc, in_maps, core_ids=list(range(B)),
                                        trace=trace)
    _last_exec_time_ns = r.exec_time_ns
    return np.stack([np.asarray(r.results[b]["o"]).astype(np.float32)
                     for b in range(B)])
